# revision 49
# baseline (speedup 1.0000x reference)
"""DiffAttention TRN2 kernel v3: 8-way (batch x seq-half) sharded, zero collectives.

v3 pipeline (ACT-exp roofline ~2.15ms/core; v2 was PE-bound at 3.03ms):
  - All matmul operands bf16 (fp32 runs 2-pass fp32_mode=HIGH at ~2x cost).
  - Phase A: qkv projections -> DRAM scratch (Q^T, K^T bf16, V bf16).
  - Phase B pair-centric: pair = (head h, branches 0/1). Q^T/K^T 128-row
    blocks naturally hold 4 combos in 4 32-partition strips; S matmuls are
    K=32 row-tiled (tile_position=(32s,0)) so strip MMs run concurrently.
    3-slot groups -> S into 3 PSUM banks -> one [128,1536] exp ACT (bf16
    out), ping-pong 3+3 banks; PV accumulates per-combo [65,512] (65th
    row = ones col of V_aug -> softmax denominators) in 2 more banks.
    Combine (u = o1 - lam*z1/z2*o2, arg = mean(u^2)+eps*z1^2) uses DVE rows
    + GPSIMD partition_broadcast/all_reduce; u kept in SBUF (ustore).
  - Phase C: rsqrt rows + sel-matmul broadcast -> onstore bf16.
  - Phase D: proj bf16, K=128 per chunk, bias via host-tiled broadcast.
"""

import sys

import numpy as np

for p in ("/opt/trn_rl_repo",):
    if p not in sys.path:
        sys.path.insert(0, p)

import ml_dtypes

import concourse.bass as bass
import concourse.bacc as bacc_mod
import concourse.bass_isa as bass_isa
import concourse.mybir as mybir
from concourse.bass_utils import run_bass_kernel_spmd
from concourse.tile import TileContext

F32 = mybir.dt.float32
F32R = mybir.dt.float32r
BF16 = mybir.dt.bfloat16

B, N, DIM, H, HD = 4, 4096, 1024, 16, 32
VD = 2 * HD  # 64, per-head v dim
NQ = 2048  # query rows per core
KT = N // 128  # 32 key tiles
CIN = DIM // 128  # 8 contraction tiles
NCORES = 8
LAMBDA_INIT = 0.2
EPS = 1e-5
SCALE = HD ** -0.5

_CACHE = {}


def build_nc(lam: float):
    nc = bacc_mod.Bacc(None, target_bir_lowering=False)

    xbt = nc.declare_dram_parameter("xbt", [DIM, N], F32, isOutput=False)
    wqkvt = nc.declare_dram_parameter("wqkvt", [DIM, 3 * DIM], BF16,
                                      isOutput=False)
    wpbt = nc.declare_dram_parameter("wpbt", [128, CIN * DIM], BF16,
                                     isOutput=False)
    biasbc = nc.declare_dram_parameter("biasbc", [128, DIM], F32,
                                       isOutput=False)
    weff = nc.declare_dram_parameter("weff", [128, 1], F32, isOutput=False)
    selp = nc.declare_dram_parameter("selp", [128, H * 128], BF16,
                                     isOutput=False)
    y = nc.declare_dram_parameter("y", [NQ, DIM], F32, isOutput=True)

    qt_s = nc.dram_tensor("qt_scratch", [DIM, NQ], BF16)
    kt_s = nc.dram_tensor("kt_scratch", [DIM, N], BF16)
    v_s = nc.dram_tensor("v_scratch", [N, DIM], BF16)

    with nc.allow_low_precision(reason="bf16 matmul operands; bf16 stores"), \
         TileContext(nc) as tc:
        with (
            tc.tile_pool(name="const", bufs=1) as constp,
            tc.tile_pool(name="store", bufs=1) as storep,
        ):
            weff_t = constp.tile([128, 1], F32)
            nc.sync.dma_start(out=weff_t, in_=weff[:, :])

            # persistent stores
            argall = storep.tile([H, NQ], F32)
            ustore = storep.tile([128, CIN, NQ], BF16)
            onstore = storep.tile([128, CIN, NQ], BF16)

            # ================= phase A: qkv (bf16) =================
            with (
                tc.tile_pool(name="xbt_p", bufs=2) as xbtp,
                tc.tile_pool(name="wq_p", bufs=1) as wqp,
                tc.tile_pool(name="drain_p", bufs=3) as drp,
                tc.tile_pool(name="psA", bufs=3, space="PSUM") as psA,
                tc.tile_pool(name="psAv", bufs=2, space="PSUM") as psAv,
            ):
                # preload all qkv weights once (per-ci DMAs stall the MMs)
                w_all = wqp.tile([128, CIN, 3 * DIM], BF16)
                nc.sync.dma_start(
                    out=w_all,
                    in_=wqkvt[:, :].rearrange("(c p) n -> p c n", p=128),
                )
                for tq in range(4):  # token quarters of 1024
                    xb = xbtp.tile([128, CIN, 1024], BF16, tag="xb")
                    for h2 in range(2):  # halve the fp32 staging footprint
                        xbf = xbtp.tile([128, CIN, 512], F32, tag="xbf")
                        nc.sync.dma_start(
                            out=xbf,
                            in_=xbt[:, tq * 1024 + h2 * 512:
                                    tq * 1024 + (h2 + 1) * 512]
                            .rearrange("(t p) n -> p t n", p=128),
                        )
                        nc.vector.tensor_copy(
                            xb[:, :, h2 * 512:(h2 + 1) * 512], xbf)
                    for co in range(2 * CIN):  # 0..7 Q, 8..15 K
                        is_q = co < CIN
                        if is_q and tq >= 2:
                            continue
                        ps = psA.tile([128, 1024], F32, tag="ps")
                        for ci in range(CIN):
                            for sb in range(2):
                                nc.tensor.matmul(
                                    ps[:, sb * 512:(sb + 1) * 512],
                                    w_all[:, ci,
                                          co * 128:(co + 1) * 128],
                                    xb[:, ci, sb * 512:(sb + 1) * 512],
                                    start=(ci == 0),
                                    stop=(ci == CIN - 1),
                                )
                        dr = drp.tile([128, 1024], BF16, tag="dr")
                        nc.vector.tensor_copy(dr, ps)
                        dst = qt_s if is_q else kt_s
                        coo = co if is_q else co - CIN
                        nc.sync.dma_start(
                            out=dst[coo * 128:(coo + 1) * 128,
                                    tq * 1024:(tq + 1) * 1024],
                            in_=dr,
                        )
                    for cc in range(DIM // 512):
                        for kt in range(8):
                            psv = psAv.tile([128, 512], F32, tag="psv")
                            for ci in range(CIN):
                                nc.tensor.matmul(
                                    psv,
                                    xb[:, ci, kt * 128:(kt + 1) * 128],
                                    w_all[:, ci, 2 * DIM + cc * 512:
                                          2 * DIM + (cc + 1) * 512],
                                    start=(ci == 0),
                                    stop=(ci == CIN - 1),
                                )
                            drv = drp.tile([128, 512], BF16, tag="drv")
                            if kt % 2 == 0:
                                nc.vector.tensor_copy(drv, psv)
                            else:
                                nc.scalar.activation(
                                    drv, psv,
                                    mybir.ActivationFunctionType.Copy)
                            nc.sync.dma_start(
                                out=v_s[tq * 1024 + kt * 128:
                                        tq * 1024 + (kt + 1) * 128,
                                        cc * 512:(cc + 1) * 512],
                                in_=drv,
                            )

            # ================= phase B: attention =================
            # slots per (head h, qb): slot i = (br=i%2, kt=i//2), strip i%4
            # (K/Q strips duplicated at partitions 64-127 so both slots of a
            #  group hit distinct 32-row PE groups -> fully concurrent S)
            # 2-slot groups x 3 PSUM regions: 3 exp-groups in flight so ACT
            # exps and DVE (Schraudolph) exps genuinely overlap. PE duty is
            # kept high (HAM warm) by folding phase-C work into the B stream.
            GSL = 2  # slots per group
            slots_hq = [(i % 2, i // 2, i % 4) for i in range(2 * KT)]
            GPQ = len(slots_hq) // GSL  # 32 groups, exact
            groups_hq = [slots_hq[GSL * i:GSL * (i + 1)] for i in range(GPQ)]
            # groups whose exp runs on DVE (Schraudolph bf16) instead of ACT
            OFFLOAD_GS = tuple(g for g in range(GPQ) if g % 3 == 1 and g < 31)
            A16 = 128.0 / np.log(2.0)
            B16 = 16250.375

            with (
                tc.tile_pool(name="kf_p", bufs=2) as kfp,
                tc.tile_pool(name="qp_p", bufs=2) as qpp,
                tc.tile_pool(name="vh_p", bufs=2) as vhp,
                tc.tile_pool(name="exp_p", bufs=6) as expp,
                tc.tile_pool(name="cp_p", bufs=2) as cpp,
                tc.tile_pool(name="row_p", bufs=2) as rowp,
                tc.tile_pool(name="sd_p", bufs=2) as sdp,
                tc.tile_pool(name="bc_p", bufs=2) as bcp,
                tc.tile_pool(name="psS", bufs=3, space="PSUM") as psS,
                tc.tile_pool(name="psO", bufs=2, space="PSUM") as psO,
            ):
                st = {}
                sched = {}

                def emit_setup_h(h, qb):
                    # K^T/Q^T 64-row head block duplicated to both halves
                    kf = kfp.tile([128, KT, 128], BF16, tag="kf")
                    qp = qpp.tile([128, 512], BF16, tag="qp")
                    for half in range(2):
                        nc.sync.dma_start(
                            out=kf[64 * half:64 * half + 64, :, :],
                            in_=kt_s[h * 64:(h + 1) * 64, :]
                            .rearrange("p (k t) -> p k t", t=128),
                        )
                        nc.sync.dma_start(
                            out=qp[64 * half:64 * half + 64, :],
                            in_=qt_s[h * 64:(h + 1) * 64,
                                     qb * 512:(qb + 1) * 512])
                    vh = vhp.tile([128, KT, 65], BF16, tag="vh")
                    nc.sync.dma_start(
                        out=vh[:, :, 0:VD],
                        in_=v_s[:, h * VD:(h + 1) * VD]
                        .rearrange("(k p) v -> p k v", p=128),
                    )
                    nc.vector.memset(vh[:, :, VD:65], 1.0)
                    st[("kf", h)], st[("qp", h)], st[("vh", h)] = kf, qp, vh

                def emit_S(h, qb, g, gidx):
                    if g == 0:
                        emit_setup_h(h, qb)
                    kf, qp = st[("kf", h)], st[("qp", h)]
                    sps = psS.tile([128, GSL, 512], F32, tag="s")
                    for j, (br, kt, s) in enumerate(groups_hq[g]):
                        nc.tensor.matmul(
                            sps[:, j, :],
                            kf[32 * s:32 * s + 32, kt, :],
                            qp[32 * s:32 * s + 32, :],
                            start=True, stop=True,
                            tile_position=(32 * s, 0),
                        )
                    n = len(groups_hq[g])
                    if g in OFFLOAD_GS:
                        ei = expp.tile([128, GSL, 512], mybir.dt.int16,
                                       tag="e", name="ei")
                        nc.vector.tensor_scalar(
                            out=ei[:, 0:n, :], in0=sps[:, 0:n, :],
                            scalar1=float(A16 * SCALE), scalar2=float(B16),
                            op0=mybir.AluOpType.mult,
                            op1=mybir.AluOpType.add)
                        st[("ex", gidx)] = ei.bitcast(BF16)
                    else:
                        ex = expp.tile([128, GSL, 512], BF16, tag="e",
                                       name="ex")
                        nc.scalar.activation(
                            ex[:, 0:n, :], sps[:, 0:n, :],
                            mybir.ActivationFunctionType.Exp, scale=SCALE,
                        )
                        st[("ex", gidx)] = ex

                def emit_PV(h, qb, g, gidx):
                    vh = st[("vh", h)]
                    ex = st.pop(("ex", gidx))
                    for j, (br, kt, s) in enumerate(groups_hq[g]):
                        if kt == 0:
                            st[("o", br)] = psO.tile(
                                [65, 512], F32, tag="o", name=f"o{br}")
                        nc.tensor.matmul(
                            st[("o", br)],
                            vh[:, kt, 0:65],
                            ex[:, j, :],
                            start=(kt == 0),
                            stop=(kt == KT - 1),
                        )
                    if g == GPQ - 1:
                        emit_drain(h, qb, gidx)

                def emit_drain(h, qb, gidx):
                    # full-tile copies free the 2 PV banks ASAP; ACT does
                    # them so the release is deterministic (DVE queue jitter
                    # stalled the next block's PV by ~4us at p99)
                    o1p = st.pop(("o", 0))
                    o2p = st.pop(("o", 1))
                    sl = slice(qb * 512, (qb + 1) * 512)
                    cp_o1 = cpp.tile([65, 512], F32, tag="co1", name="co1")
                    nc.scalar.activation(
                        cp_o1, o1p, mybir.ActivationFunctionType.Copy)
                    cp_o2 = cpp.tile([65, 512], F32, tag="co2", name="co2")
                    nc.scalar.activation(
                        cp_o2, o2p, mybir.ActivationFunctionType.Copy)
                    # z rows to partition 0 (DVE is lane-locked; DMA moves)
                    z1 = rowp.tile([1, 512], F32, tag="z1", name="z1")
                    nc.sync.dma_start(out=z1, in_=cp_o1[VD:65, :])
                    z2 = rowp.tile([1, 512], F32, tag="z2", name="z2")
                    nc.sync.dma_start(out=z2, in_=cp_o2[VD:65, :])

                    def part1(h=h, sl=sl, cp_o1=cp_o1, cp_o2=cp_o2,
                              z1=z1, z2=z2):
                        rzs = rowp.tile([1, 512], F32, tag="rzs", name="rzs")
                        rz2 = rowp.tile([1, 512], F32, tag="rz2", name="rz2")
                        nc.vector.reciprocal_approx_accurate(
                            out=rz2, in_=z2, scratch=rzs)
                        trow = rowp.tile([1, 512], F32, tag="tr", name="tr")
                        nc.vector.scalar_tensor_tensor(
                            out=trow, in0=z1, scalar=float(lam), in1=rz2,
                            op0=mybir.AluOpType.mult,
                            op1=mybir.AluOpType.mult)
                        tbc = cpp.tile([VD, 512], F32, tag="tbc", name="tbc")
                        nc.gpsimd.partition_broadcast(tbc, trow)
                        st[("t", h, sl.start)] = (tbc, cp_o1, cp_o2, z1)

                    def part2(h=h, qb=qb, sl=sl):
                        tbc, cp_o1, cp_o2, z1 = st.pop(("t", h, sl.start))
                        x1 = cpp.tile([VD, 512], F32, tag="x1", name="x1")
                        nc.vector.tensor_mul(x1, cp_o2[0:VD, :], tbc)
                        u_t = cpp.tile([VD, 512], BF16, tag="ut", name="ut")
                        nc.vector.tensor_sub(u_t, cp_o1[0:VD, :], x1)
                        rsl = slice((h % 2) * VD, (h % 2) * VD + VD)
                        nc.sync.dma_start(
                            out=ustore[rsl, h // 2, sl], in_=u_t)
                        u2 = cpp.tile([VD, 512], F32, tag="u2", name="u2")
                        nc.vector.tensor_mul(u2, u_t, u_t)
                        s2 = cpp.tile([VD, 512], F32, tag="s2", name="s2")
                        nc.gpsimd.partition_all_reduce(
                            s2, u2, channels=VD,
                            reduce_op=bass_isa.ReduceOp.add)
                        st[("u", h, sl.start)] = (s2, z1)

                    def part3(h=h, sl=sl):
                        s2, z1 = st.pop(("u", h, sl.start))
                        ze = rowp.tile([1, 512], F32, tag="ze", name="ze")
                        nc.vector.tensor_scalar_mul(
                            ze, z1, float(EPS ** 0.5))
                        zsq = rowp.tile([1, 512], F32, tag="zq", name="zq")
                        nc.vector.tensor_mul(zsq, ze, ze)
                        arg0 = rowp.tile([1, 512], F32, tag="ar", name="ar")
                        nc.vector.scalar_tensor_tensor(
                            out=arg0, in0=s2[0:1, :],
                            scalar=1.0 / VD, in1=zsq,
                            op0=mybir.AluOpType.mult,
                            op1=mybir.AluOpType.add)
                        nc.sync.dma_start(out=argall[h:h + 1, sl], in_=arg0)

                    sched.setdefault(gidx + 2, []).append(part1)
                    sched.setdefault(gidx + 5, []).append(part2)
                    sched.setdefault(gidx + 8, []).append(part3)

                    if h == H - 1:
                        # whole qb-block drained ~8 groups from now: fold its
                        # phase-C (norm-scale) into the next block's stream
                        def rows_fn(qb=qb, sl=sl):
                            sd = sdp.tile([H, 512], F32, tag="sd", name="sd")
                            nc.scalar.activation(
                                sd, argall[:, sl],
                                mybir.ActivationFunctionType.Sqrt)
                            scr = sdp.tile([H, 512], F32, tag="sc",
                                           name="sc")
                            rrb = sdp.tile([H, 512], F32, tag="rr",
                                           name="rr")
                            nc.vector.reciprocal_approx_accurate(
                                out=rrb, in_=sd, scratch=scr)
                            st[("rrb", qb)] = rrb

                        sched.setdefault(gidx + 10, []).append(rows_fn)
                        for k in range(H):
                            def c_fn(k=k, qb=qb, sl=sl):
                                rrb = st[("rrb", qb)]
                                rowq = rowp.tile([1, 512], F32, tag="rq",
                                                 name="rq")
                                nc.sync.dma_start(
                                    out=rowq, in_=rrb[k:k + 1, :])
                                tbc_rr = bcp.tile([128, 512], F32,
                                                  tag="bc", name="bc")
                                nc.gpsimd.partition_broadcast(tbc_rr, rowq)
                                rsl = slice((k % 2) * VD, (k % 2) * VD + VD)
                                onf = bcp.tile([128, 512], F32, tag="on",
                                               name="on")
                                nc.vector.tensor_mul(
                                    onf[rsl, :], ustore[rsl, k // 2, sl],
                                    tbc_rr[rsl, :])
                                nc.vector.tensor_scalar_mul(
                                    onstore[rsl, k // 2, sl], onf[rsl, :],
                                    weff_t[rsl, :])
                                if k == H - 1:
                                    st.pop(("rrb", qb))

                            sched.setdefault(gidx + 12 + 2 * k,
                                             []).append(c_fn)

                items = [(h, qb, g)
                         for qb in range(4)
                         for h in range(H)
                         for g in range(GPQ)]
                LOOK = 3
                for j in range(LOOK):
                    emit_S(*items[j], j)
                for gidx in range(len(items)):
                    if gidx + LOOK < len(items):
                        emit_S(*items[gidx + LOOK], gidx + LOOK)
                    emit_PV(*items[gidx], gidx)
                    for fn in sched.pop(gidx, []):
                        fn()
                for kk in sorted(sched):
                    for fn in sched[kk]:
                        fn()

            # ============ phase D: proj (bf16, K=128) ============
            # (phase C is folded into the B stream per qb block)
            with (
                tc.tile_pool(name="wp_p", bufs=1) as wpp,
                tc.tile_pool(name="yd_p", bufs=3) as ydp,
                tc.tile_pool(name="psY", bufs=2, space="PSUM") as psY,
            ):
                wpb = wpp.tile([128, CIN, DIM], BF16)
                nc.sync.dma_start(
                    out=wpb,
                    in_=wpbt[:, :].rearrange("v (c n) -> v c n", c=CIN))
                bb = wpp.tile([128, DIM], F32)
                nc.sync.dma_start(out=bb, in_=biasbc[:, :])
                for qt in range(NQ // 128):
                    yps = psY.tile([128, 1024], F32, tag="y")
                    for sb in range(2):
                        for ci in range(CIN):
                            nc.tensor.matmul(
                                yps[:, sb * 512:(sb + 1) * 512],
                                onstore[:, ci, qt * 128:(qt + 1) * 128],
                                wpb[:, ci, sb * 512:(sb + 1) * 512],
                                start=(ci == 0),
                                stop=(ci == CIN - 1),
                            )
                    yd = ydp.tile([128, 1024], F32, tag="yd")
                    nc.vector.tensor_add(yd, yps, bb)
                    nc.sync.dma_start(
                        out=y[qt * 128:(qt + 1) * 128, :], in_=yd)
    nc.finalize()
    return nc


def _make_inputs(x, w_qkv, w_proj, b_proj, sub_norm_w):
    wqkvt = np.ascontiguousarray(
        np.asarray(w_qkv, np.float32).T).astype(ml_dtypes.bfloat16)
    wprojt = np.ascontiguousarray(np.asarray(w_proj, np.float32).T)  # [c, out]
    # proj weights: partition (h%2)*64+vd, col (h//2)*DIM+out
    wpbt = np.ascontiguousarray(
        wprojt.reshape(CIN, 2, VD, DIM).transpose(1, 2, 0, 3)
        .reshape(128, CIN * DIM)).astype(ml_dtypes.bfloat16)
    biasbc = np.ascontiguousarray(
        np.tile(np.asarray(b_proj, np.float32).reshape(1, DIM), (128, 1)))
    # selfat[:, h, :]: [128,128]; row h one-hot -> cols (h%2)*64..+64
    selp = np.zeros((128, H, 128), np.float32)
    for h in range(H):
        po = (h % 2) * VD
        selp[h, h, po:po + VD] = 1.0
    selp = np.ascontiguousarray(
        selp.reshape(128, H * 128)).astype(ml_dtypes.bfloat16)
    weff = np.tile(
        (np.asarray(sub_norm_w, np.float32)
         * (1.0 - LAMBDA_INIT)).reshape(VD, 1), (2, 1))
    weff = np.ascontiguousarray(weff)
    return wqkvt, wpbt, biasbc, weff, selp


def _in_maps(inputs):
    x = np.asarray(inputs["x"], np.float32)
    wqkvt, wpbt, biasbc, weff, selp = _make_inputs(
        x, inputs["w_qkv"], inputs["w_proj"], inputs["b_proj"],
        inputs["sub_norm_w"])
    in_maps = []
    for c in range(NCORES):
        b, half = c // 2, c % 2
        xt = np.asarray(x[b].T)  # [DIM, N]
        if half == 1:  # query rows first
            xt = np.concatenate([xt[:, NQ:], xt[:, :NQ]], axis=1)
        in_maps.append({
            "xbt": np.ascontiguousarray(xt),
            "wqkvt": wqkvt,
            "wpbt": wpbt,
            "biasbc": biasbc,
            "weff": weff,
            "selp": selp,
        })
    return in_maps


def kernel(x, w_qkv, w_proj, b_proj, lambda_q1, lambda_k1, lambda_q2,
           lambda_k2, sub_norm_w):
    lam = float(
        np.exp(np.sum(np.float64(lambda_q1) * np.float64(lambda_k1)))
        - np.exp(np.sum(np.float64(lambda_q2) * np.float64(lambda_k2)))
        + LAMBDA_INIT
    )

    key = round(lam, 12)
    if key not in _CACHE:
        _CACHE[key] = build_nc(lam)
    nc = _CACHE[key]

    in_maps = _in_maps(dict(
        x=x, w_qkv=w_qkv, w_proj=w_proj, b_proj=b_proj, sub_norm_w=sub_norm_w))
    res = run_bass_kernel_spmd(nc, in_maps, list(range(NCORES)))
    out = np.empty((B, N, DIM), np.float32)
    for c in range(NCORES):
        b, half = c // 2, c % 2
        out[b, half * NQ:(half + 1) * NQ, :] = res.results[c]["y"]
    return out


# revision 52
# speedup vs baseline: 1.0608x; 1.0608x over previous
"""DiffAttention TRN2 kernel v3: 8-way (batch x seq-half) sharded, zero collectives.

v3 pipeline (ACT-exp roofline ~2.15ms/core; v2 was PE-bound at 3.03ms):
  - All matmul operands bf16 (fp32 runs 2-pass fp32_mode=HIGH at ~2x cost).
  - Phase A: qkv projections -> DRAM scratch (Q^T, K^T bf16, V bf16).
  - Phase B pair-centric: pair = (head h, branches 0/1). Q^T/K^T 128-row
    blocks naturally hold 4 combos in 4 32-partition strips; S matmuls are
    K=32 row-tiled (tile_position=(32s,0)) so strip MMs run concurrently.
    3-slot groups -> S into 3 PSUM banks -> one [128,1536] exp ACT (bf16
    out), ping-pong 3+3 banks; PV accumulates per-combo [65,512] (65th
    row = ones col of V_aug -> softmax denominators) in 2 more banks.
    Combine (u = o1 - lam*z1/z2*o2, arg = mean(u^2)+eps*z1^2) uses DVE rows
    + GPSIMD partition_broadcast/all_reduce; u kept in SBUF (ustore).
  - Phase C: rsqrt rows + sel-matmul broadcast -> onstore bf16.
  - Phase D: proj bf16, K=128 per chunk, bias via host-tiled broadcast.
"""

import sys

import numpy as np

for p in ("/opt/trn_rl_repo",):
    if p not in sys.path:
        sys.path.insert(0, p)

import ml_dtypes

import concourse.bass as bass
import concourse.bacc as bacc_mod
import concourse.bass_isa as bass_isa
import concourse.mybir as mybir
from concourse.bass_utils import run_bass_kernel_spmd
from concourse.tile import TileContext

F32 = mybir.dt.float32
F32R = mybir.dt.float32r
BF16 = mybir.dt.bfloat16

B, N, DIM, H, HD = 4, 4096, 1024, 16, 32
VD = 2 * HD  # 64, per-head v dim
NQ = 2048  # query rows per core
KT = N // 128  # 32 key tiles
CIN = DIM // 128  # 8 contraction tiles
NCORES = 8
LAMBDA_INIT = 0.2
EPS = 1e-5
SCALE = HD ** -0.5

_CACHE = {}


def build_nc(lam: float):
    nc = bacc_mod.Bacc(None, target_bir_lowering=False)

    xbt = nc.declare_dram_parameter("xbt", [DIM, N], F32, isOutput=False)
    wqkvt = nc.declare_dram_parameter("wqkvt", [DIM, 3 * DIM], BF16,
                                      isOutput=False)
    wpbt = nc.declare_dram_parameter("wpbt", [128, CIN * DIM], BF16,
                                     isOutput=False)
    biasbc = nc.declare_dram_parameter("biasbc", [128, DIM], F32,
                                       isOutput=False)
    weff = nc.declare_dram_parameter("weff", [128, 1], F32, isOutput=False)
    selp = nc.declare_dram_parameter("selp", [128, H * 128], BF16,
                                     isOutput=False)
    y = nc.declare_dram_parameter("y", [NQ, DIM], F32, isOutput=True)

    qt_s = nc.dram_tensor("qt_scratch", [DIM, NQ], BF16)
    kt_s = nc.dram_tensor("kt_scratch", [DIM, N], BF16)
    v_s = nc.dram_tensor("v_scratch", [N, DIM], BF16)

    with nc.allow_low_precision(reason="bf16 matmul operands; bf16 stores"), \
         TileContext(nc) as tc:
        with (
            tc.tile_pool(name="const", bufs=1) as constp,
            tc.tile_pool(name="store", bufs=1) as storep,
        ):
            weff_t = constp.tile([128, 1], F32)
            nc.sync.dma_start(out=weff_t, in_=weff[:, :])

            # persistent stores
            argall = storep.tile([H, NQ], F32)
            ustore = storep.tile([128, CIN, NQ], BF16)
            onstore = storep.tile([128, CIN, NQ], BF16)

            # ================= phase A: qkv (bf16) =================
            with (
                tc.tile_pool(name="xbt_p", bufs=2) as xbtp,
                tc.tile_pool(name="wq_p", bufs=1) as wqp,
                tc.tile_pool(name="drain_p", bufs=3) as drp,
                tc.tile_pool(name="psA", bufs=3, space="PSUM") as psA,
                tc.tile_pool(name="psAv", bufs=2, space="PSUM") as psAv,
            ):
                # preload all qkv weights once (per-ci DMAs stall the MMs)
                w_all = wqp.tile([128, CIN, 3 * DIM], BF16)
                nc.sync.dma_start(
                    out=w_all,
                    in_=wqkvt[:, :].rearrange("(c p) n -> p c n", p=128),
                )
                for tq in range(4):  # token quarters of 1024
                    xb = xbtp.tile([128, CIN, 1024], BF16, tag="xb")
                    for h2 in range(2):  # halve the fp32 staging footprint
                        xbf = xbtp.tile([128, CIN, 512], F32, tag="xbf")
                        nc.sync.dma_start(
                            out=xbf,
                            in_=xbt[:, tq * 1024 + h2 * 512:
                                    tq * 1024 + (h2 + 1) * 512]
                            .rearrange("(t p) n -> p t n", p=128),
                        )
                        nc.vector.tensor_copy(
                            xb[:, :, h2 * 512:(h2 + 1) * 512], xbf)
                    for co in range(2 * CIN):  # 0..7 Q, 8..15 K
                        is_q = co < CIN
                        if is_q and tq >= 2:
                            continue
                        ps = psA.tile([128, 1024], F32, tag="ps")
                        for ci in range(CIN):
                            for sb in range(2):
                                nc.tensor.matmul(
                                    ps[:, sb * 512:(sb + 1) * 512],
                                    w_all[:, ci,
                                          co * 128:(co + 1) * 128],
                                    xb[:, ci, sb * 512:(sb + 1) * 512],
                                    start=(ci == 0),
                                    stop=(ci == CIN - 1),
                                )
                        dr = drp.tile([128, 1024], BF16, tag="dr")
                        nc.vector.tensor_copy(dr, ps)
                        dst = qt_s if is_q else kt_s
                        coo = co if is_q else co - CIN
                        nc.sync.dma_start(
                            out=dst[coo * 128:(coo + 1) * 128,
                                    tq * 1024:(tq + 1) * 1024],
                            in_=dr,
                        )
                    for cc in range(DIM // 512):
                        for kt in range(8):
                            psv = psAv.tile([128, 512], F32, tag="psv")
                            for ci in range(CIN):
                                nc.tensor.matmul(
                                    psv,
                                    xb[:, ci, kt * 128:(kt + 1) * 128],
                                    w_all[:, ci, 2 * DIM + cc * 512:
                                          2 * DIM + (cc + 1) * 512],
                                    start=(ci == 0),
                                    stop=(ci == CIN - 1),
                                )
                            drv = drp.tile([128, 512], BF16, tag="drv")
                            if kt % 2 == 0:
                                nc.vector.tensor_copy(drv, psv)
                            else:
                                nc.scalar.activation(
                                    drv, psv,
                                    mybir.ActivationFunctionType.Copy)
                            nc.sync.dma_start(
                                out=v_s[tq * 1024 + kt * 128:
                                        tq * 1024 + (kt + 1) * 128,
                                        cc * 512:(cc + 1) * 512],
                                in_=drv,
                            )

            # ================= phase B: attention =================
            # slots per (head h, qb): slot i = (br=i%2, kt=i//2), strip i%4
            # (K/Q strips duplicated at partitions 64-127 so both slots of a
            #  group hit distinct 32-row PE groups -> fully concurrent S)
            # 2-slot groups x 3 PSUM regions: 3 exp-groups in flight so ACT
            # exps and DVE (Schraudolph) exps genuinely overlap. PE duty is
            # kept high (HAM warm) by folding phase-C work into the B stream.
            GSL = 3  # slots per group (3-slot keeps PE duty high: HAM warm)
            slots_hq = [(i % 2, i // 2, i % 4) for i in range(2 * KT)]
            GPQ = (len(slots_hq) + GSL - 1) // GSL  # 22 (last has 1 slot)
            groups_hq = [slots_hq[GSL * i:GSL * (i + 1)] for i in range(GPQ)]
            # groups whose exp runs on DVE (Schraudolph bf16) instead of ACT
            OFFLOAD_GS = (1, 5, 9, 13, 17)
            A16 = 128.0 / np.log(2.0)
            B16 = 16250.375

            with (
                tc.tile_pool(name="kf_p", bufs=2) as kfp,
                tc.tile_pool(name="qp_p", bufs=2) as qpp,
                tc.tile_pool(name="vh_p", bufs=2) as vhp,
                tc.tile_pool(name="exp_p", bufs=6) as expp,
                tc.tile_pool(name="cp_p", bufs=2) as cpp,
                tc.tile_pool(name="row_p", bufs=2) as rowp,
                tc.tile_pool(name="sd_p", bufs=2) as sdp,
                tc.tile_pool(name="bc_p", bufs=2) as bcp,
                tc.tile_pool(name="psS", bufs=2, space="PSUM") as psS,
                tc.tile_pool(name="psO", bufs=2, space="PSUM") as psO,
            ):
                st = {}
                sched = {}

                def emit_setup_h(h, qb):
                    # K^T/Q^T 64-row head block duplicated to both halves
                    kf = kfp.tile([128, KT, 128], BF16, tag="kf")
                    qp = qpp.tile([128, 512], BF16, tag="qp")
                    for half in range(2):
                        nc.sync.dma_start(
                            out=kf[64 * half:64 * half + 64, :, :],
                            in_=kt_s[h * 64:(h + 1) * 64, :]
                            .rearrange("p (k t) -> p k t", t=128),
                        )
                        nc.sync.dma_start(
                            out=qp[64 * half:64 * half + 64, :],
                            in_=qt_s[h * 64:(h + 1) * 64,
                                     qb * 512:(qb + 1) * 512])
                    vh = vhp.tile([128, KT, 65], BF16, tag="vh")
                    nc.sync.dma_start(
                        out=vh[:, :, 0:VD],
                        in_=v_s[:, h * VD:(h + 1) * VD]
                        .rearrange("(k p) v -> p k v", p=128),
                    )
                    nc.vector.memset(vh[:, :, VD:65], 1.0)
                    st[("kf", h)], st[("qp", h)], st[("vh", h)] = kf, qp, vh

                def emit_S(h, qb, g, gidx):
                    if g == 0:
                        emit_setup_h(h, qb)
                    kf, qp = st[("kf", h)], st[("qp", h)]
                    sps = psS.tile([128, GSL, 512], F32, tag="s")
                    for j, (br, kt, s) in enumerate(groups_hq[g]):
                        nc.tensor.matmul(
                            sps[:, j, :],
                            kf[32 * s:32 * s + 32, kt, :],
                            qp[32 * s:32 * s + 32, :],
                            start=True, stop=True,
                            tile_position=(32 * s, 0),
                        )
                    n = len(groups_hq[g])
                    if g in OFFLOAD_GS:
                        ei = expp.tile([128, GSL, 512], mybir.dt.int16,
                                       tag="e", name="ei")
                        nc.vector.tensor_scalar(
                            out=ei[:, 0:n, :], in0=sps[:, 0:n, :],
                            scalar1=float(A16 * SCALE), scalar2=float(B16),
                            op0=mybir.AluOpType.mult,
                            op1=mybir.AluOpType.add)
                        st[("ex", gidx)] = ei.bitcast(BF16)
                    else:
                        ex = expp.tile([128, GSL, 512], BF16, tag="e",
                                       name="ex")
                        nc.scalar.activation(
                            ex[:, 0:n, :], sps[:, 0:n, :],
                            mybir.ActivationFunctionType.Exp, scale=SCALE,
                        )
                        st[("ex", gidx)] = ex

                def emit_PV(h, qb, g, gidx):
                    vh = st[("vh", h)]
                    ex = st.pop(("ex", gidx))
                    for j, (br, kt, s) in enumerate(groups_hq[g]):
                        if kt == 0:
                            st[("o", br)] = psO.tile(
                                [65, 512], F32, tag="o", name=f"o{br}")
                        nc.tensor.matmul(
                            st[("o", br)],
                            vh[:, kt, 0:65],
                            ex[:, j, :],
                            start=(kt == 0),
                            stop=(kt == KT - 1),
                        )
                    if g == GPQ - 1:
                        emit_drain(h, qb, gidx)

                def emit_drain(h, qb, gidx):
                    # full-tile copies free the 2 PV banks ASAP; ACT does
                    # them so the release is deterministic (DVE queue jitter
                    # stalled the next block's PV by ~4us at p99)
                    o1p = st.pop(("o", 0))
                    o2p = st.pop(("o", 1))
                    sl = slice(qb * 512, (qb + 1) * 512)
                    cp_o1 = cpp.tile([65, 512], F32, tag="co1", name="co1")
                    nc.scalar.activation(
                        cp_o1, o1p, mybir.ActivationFunctionType.Copy)
                    cp_o2 = cpp.tile([65, 512], F32, tag="co2", name="co2")
                    nc.scalar.activation(
                        cp_o2, o2p, mybir.ActivationFunctionType.Copy)
                    # z rows to partition 0 (DVE is lane-locked; DMA moves)
                    z1 = rowp.tile([1, 512], F32, tag="z1", name="z1")
                    nc.sync.dma_start(out=z1, in_=cp_o1[VD:65, :])
                    z2 = rowp.tile([1, 512], F32, tag="z2", name="z2")
                    nc.sync.dma_start(out=z2, in_=cp_o2[VD:65, :])

                    def part1(h=h, sl=sl, cp_o1=cp_o1, cp_o2=cp_o2,
                              z1=z1, z2=z2):
                        rzs = rowp.tile([1, 512], F32, tag="rzs", name="rzs")
                        rz2 = rowp.tile([1, 512], F32, tag="rz2", name="rz2")
                        nc.vector.reciprocal_approx_accurate(
                            out=rz2, in_=z2, scratch=rzs)
                        trow = rowp.tile([1, 512], F32, tag="tr", name="tr")
                        nc.vector.scalar_tensor_tensor(
                            out=trow, in0=z1, scalar=float(lam), in1=rz2,
                            op0=mybir.AluOpType.mult,
                            op1=mybir.AluOpType.mult)
                        tbc = cpp.tile([VD, 512], F32, tag="tbc", name="tbc")
                        nc.gpsimd.partition_broadcast(tbc, trow)
                        st[("t", h, sl.start)] = (tbc, cp_o1, cp_o2, z1)

                    def part2(h=h, qb=qb, sl=sl):
                        tbc, cp_o1, cp_o2, z1 = st.pop(("t", h, sl.start))
                        x1 = cpp.tile([VD, 512], F32, tag="x1", name="x1")
                        nc.vector.tensor_mul(x1, cp_o2[0:VD, :], tbc)
                        u_t = cpp.tile([VD, 512], BF16, tag="ut", name="ut")
                        nc.vector.tensor_sub(u_t, cp_o1[0:VD, :], x1)
                        rsl = slice((h % 2) * VD, (h % 2) * VD + VD)
                        nc.sync.dma_start(
                            out=ustore[rsl, h // 2, sl], in_=u_t)
                        u2 = cpp.tile([VD, 512], F32, tag="u2", name="u2")
                        nc.vector.tensor_mul(u2, u_t, u_t)
                        s2 = cpp.tile([VD, 512], F32, tag="s2", name="s2")
                        nc.gpsimd.partition_all_reduce(
                            s2, u2, channels=VD,
                            reduce_op=bass_isa.ReduceOp.add)
                        st[("u", h, sl.start)] = (s2, z1)

                    def part3(h=h, sl=sl):
                        s2, z1 = st.pop(("u", h, sl.start))
                        ze = rowp.tile([1, 512], F32, tag="ze", name="ze")
                        nc.vector.tensor_scalar_mul(
                            ze, z1, float(EPS ** 0.5))
                        zsq = rowp.tile([1, 512], F32, tag="zq", name="zq")
                        nc.vector.tensor_mul(zsq, ze, ze)
                        arg0 = rowp.tile([1, 512], F32, tag="ar", name="ar")
                        nc.vector.scalar_tensor_tensor(
                            out=arg0, in0=s2[0:1, :],
                            scalar=1.0 / VD, in1=zsq,
                            op0=mybir.AluOpType.mult,
                            op1=mybir.AluOpType.add)
                        nc.sync.dma_start(out=argall[h:h + 1, sl], in_=arg0)

                    sched.setdefault(gidx + 2, []).append(part1)
                    sched.setdefault(gidx + 5, []).append(part2)
                    sched.setdefault(gidx + 8, []).append(part3)

                    if h == H - 1:
                        # whole qb-block drained ~8 groups from now: fold its
                        # phase-C (norm-scale) into the next block's stream
                        def rows_fn(qb=qb, sl=sl):
                            sd = sdp.tile([H, 512], F32, tag="sd", name="sd")
                            nc.scalar.activation(
                                sd, argall[:, sl],
                                mybir.ActivationFunctionType.Sqrt)
                            scr = sdp.tile([H, 512], F32, tag="sc",
                                           name="sc")
                            rrb = sdp.tile([H, 512], F32, tag="rr",
                                           name="rr")
                            nc.vector.reciprocal_approx_accurate(
                                out=rrb, in_=sd, scratch=scr)
                            st[("rrb", qb)] = rrb

                        sched.setdefault(gidx + 10, []).append(rows_fn)
                        for k in range(H):
                            def c_fn(k=k, qb=qb, sl=sl):
                                rrb = st[("rrb", qb)]
                                rowq = rowp.tile([1, 512], F32, tag="rq",
                                                 name="rq")
                                nc.sync.dma_start(
                                    out=rowq, in_=rrb[k:k + 1, :])
                                tbc_rr = bcp.tile([128, 512], F32,
                                                  tag="bc", name="bc")
                                nc.gpsimd.partition_broadcast(tbc_rr, rowq)
                                rsl = slice((k % 2) * VD, (k % 2) * VD + VD)
                                onf = bcp.tile([128, 512], F32, tag="on",
                                               name="on")
                                nc.vector.tensor_mul(
                                    onf[rsl, :], ustore[rsl, k // 2, sl],
                                    tbc_rr[rsl, :])
                                nc.vector.tensor_scalar_mul(
                                    onstore[rsl, k // 2, sl], onf[rsl, :],
                                    weff_t[rsl, :])
                                if k == H - 1:
                                    st.pop(("rrb", qb))

                            sched.setdefault(gidx + 12 + 2 * k,
                                             []).append(c_fn)

                items = [(h, qb, g)
                         for qb in range(4)
                         for h in range(H)
                         for g in range(GPQ)]
                LOOK = 2
                for j in range(LOOK):
                    emit_S(*items[j], j)
                for gidx in range(len(items)):
                    if gidx + LOOK < len(items):
                        emit_S(*items[gidx + LOOK], gidx + LOOK)
                    emit_PV(*items[gidx], gidx)
                    for fn in sched.pop(gidx, []):
                        fn()
                for kk in sorted(sched):
                    for fn in sched[kk]:
                        fn()

            # ============ phase D: proj (bf16, K=128) ============
            # (phase C is folded into the B stream per qb block)
            with (
                tc.tile_pool(name="wp_p", bufs=1) as wpp,
                tc.tile_pool(name="yd_p", bufs=3) as ydp,
                tc.tile_pool(name="psY", bufs=2, space="PSUM") as psY,
            ):
                wpb = wpp.tile([128, CIN, DIM], BF16)
                nc.sync.dma_start(
                    out=wpb,
                    in_=wpbt[:, :].rearrange("v (c n) -> v c n", c=CIN))
                bb = wpp.tile([128, DIM], F32)
                nc.sync.dma_start(out=bb, in_=biasbc[:, :])
                for qt in range(NQ // 128):
                    yps = psY.tile([128, 1024], F32, tag="y")
                    for sb in range(2):
                        for ci in range(CIN):
                            nc.tensor.matmul(
                                yps[:, sb * 512:(sb + 1) * 512],
                                onstore[:, ci, qt * 128:(qt + 1) * 128],
                                wpb[:, ci, sb * 512:(sb + 1) * 512],
                                start=(ci == 0),
                                stop=(ci == CIN - 1),
                            )
                    yd = ydp.tile([128, 1024], F32, tag="yd")
                    nc.vector.tensor_add(yd, yps, bb)
                    nc.sync.dma_start(
                        out=y[qt * 128:(qt + 1) * 128, :], in_=yd)
    nc.finalize()
    return nc


def _make_inputs(x, w_qkv, w_proj, b_proj, sub_norm_w):
    wqkvt = np.ascontiguousarray(
        np.asarray(w_qkv, np.float32).T).astype(ml_dtypes.bfloat16)
    wprojt = np.ascontiguousarray(np.asarray(w_proj, np.float32).T)  # [c, out]
    # proj weights: partition (h%2)*64+vd, col (h//2)*DIM+out
    wpbt = np.ascontiguousarray(
        wprojt.reshape(CIN, 2, VD, DIM).transpose(1, 2, 0, 3)
        .reshape(128, CIN * DIM)).astype(ml_dtypes.bfloat16)
    biasbc = np.ascontiguousarray(
        np.tile(np.asarray(b_proj, np.float32).reshape(1, DIM), (128, 1)))
    # selfat[:, h, :]: [128,128]; row h one-hot -> cols (h%2)*64..+64
    selp = np.zeros((128, H, 128), np.float32)
    for h in range(H):
        po = (h % 2) * VD
        selp[h, h, po:po + VD] = 1.0
    selp = np.ascontiguousarray(
        selp.reshape(128, H * 128)).astype(ml_dtypes.bfloat16)
    weff = np.tile(
        (np.asarray(sub_norm_w, np.float32)
         * (1.0 - LAMBDA_INIT)).reshape(VD, 1), (2, 1))
    weff = np.ascontiguousarray(weff)
    return wqkvt, wpbt, biasbc, weff, selp


def _in_maps(inputs):
    x = np.asarray(inputs["x"], np.float32)
    wqkvt, wpbt, biasbc, weff, selp = _make_inputs(
        x, inputs["w_qkv"], inputs["w_proj"], inputs["b_proj"],
        inputs["sub_norm_w"])
    in_maps = []
    for c in range(NCORES):
        b, half = c // 2, c % 2
        xt = np.asarray(x[b].T)  # [DIM, N]
        if half == 1:  # query rows first
            xt = np.concatenate([xt[:, NQ:], xt[:, :NQ]], axis=1)
        in_maps.append({
            "xbt": np.ascontiguousarray(xt),
            "wqkvt": wqkvt,
            "wpbt": wpbt,
            "biasbc": biasbc,
            "weff": weff,
            "selp": selp,
        })
    return in_maps


def kernel(x, w_qkv, w_proj, b_proj, lambda_q1, lambda_k1, lambda_q2,
           lambda_k2, sub_norm_w):
    lam = float(
        np.exp(np.sum(np.float64(lambda_q1) * np.float64(lambda_k1)))
        - np.exp(np.sum(np.float64(lambda_q2) * np.float64(lambda_k2)))
        + LAMBDA_INIT
    )

    key = round(lam, 12)
    if key not in _CACHE:
        _CACHE[key] = build_nc(lam)
    nc = _CACHE[key]

    in_maps = _in_maps(dict(
        x=x, w_qkv=w_qkv, w_proj=w_proj, b_proj=b_proj, sub_norm_w=sub_norm_w))
    res = run_bass_kernel_spmd(nc, in_maps, list(range(NCORES)))
    out = np.empty((B, N, DIM), np.float32)
    for c in range(NCORES):
        b, half = c // 2, c % 2
        out[b, half * NQ:(half + 1) * NQ, :] = res.results[c]["y"]
    return out


# revision 55
# speedup vs baseline: 1.1808x; 1.1131x over previous
"""DiffAttention TRN2 kernel: 8-way (batch x seq-half) sharded, zero collectives.

HW-validated at 2.502ms/core (baseline 3.027ms), rel err 6.8e-3:
  - All matmul operands bf16 (fp32 runs 2-pass fp32_mode=HIGH at ~2x cost).
  - Phase A: qkv projections -> DRAM scratch (Q^T/K^T/V bf16); all weights
    preloaded to SBUF once (509 per-tile weight DMAs cost 319us of stalls).
  - Phase B, qb-block outer / head inner. Per (head, qb): 64 slots =
    (branch, key-tile); slot i at strip i%4 (K/Q 64-row head blocks
    duplicated into both SBUF halves) so the K=32 row-tiled S matmuls
    (tile_position=(32s,0)) run concurrently. 3-slot groups -> S into 3
    PSUM banks (ping-pong 3+3) -> one [128,1536] exp ACT (bf16 out); 5 of
    22 groups exp on DVE instead (1-op Schraudolph: tensor_scalar mult/add
    to int16, bitcast bf16; softmax cancels the mean bias). PV accumulates
    per-branch [65,512] (65th row = ones col of V_aug -> softmax
    denominators Z) in the last 2 banks. 3-slot groups keep PE duty ~87%
    per cycle - smaller groups drop below the HAM warm threshold and the
    PE clock halves (measured 60% throttle-active).
    Combine (u = o1 - lam*z1/z2*o2, arg = mean(u^2)+eps*z1^2): ACT drains
    the PV banks (deterministic release), DVE row math, GPSIMD
    partition_broadcast/all_reduce; u stays in SBUF (ustore).
  - Phase C folded into B: per drained qb block, sqrt+recip rows then
    per-head GPSIMD rr-broadcast * weff -> onstore bf16 (no PSUM needed).
  - Phase D: proj bf16, K=128 chunks, bias via host-tiled broadcast.
"""

import sys

import numpy as np

for p in ("/opt/trn_rl_repo",):
    if p not in sys.path:
        sys.path.insert(0, p)

import ml_dtypes

import concourse.bass as bass
import concourse.bacc as bacc_mod
import concourse.bass_isa as bass_isa
import concourse.mybir as mybir
from concourse.bass_utils import run_bass_kernel_spmd
from concourse.tile import TileContext

F32 = mybir.dt.float32
F32R = mybir.dt.float32r
BF16 = mybir.dt.bfloat16

B, N, DIM, H, HD = 4, 4096, 1024, 16, 32
VD = 2 * HD  # 64, per-head v dim
NQ = 2048  # query rows per core
KT = N // 128  # 32 key tiles
CIN = DIM // 128  # 8 contraction tiles
NCORES = 8
LAMBDA_INIT = 0.2
EPS = 1e-5
SCALE = HD ** -0.5

_CACHE = {}


def build_nc(lam: float):
    nc = bacc_mod.Bacc(None, target_bir_lowering=False)

    xbt = nc.declare_dram_parameter("xbt", [DIM, N], F32, isOutput=False)
    wqkvt = nc.declare_dram_parameter("wqkvt", [DIM, 3 * DIM], BF16,
                                      isOutput=False)
    wpbt = nc.declare_dram_parameter("wpbt", [128, CIN * DIM], BF16,
                                     isOutput=False)
    biasbc = nc.declare_dram_parameter("biasbc", [128, DIM], F32,
                                       isOutput=False)
    weff = nc.declare_dram_parameter("weff", [128, 1], F32, isOutput=False)
    selp = nc.declare_dram_parameter("selp", [128, H * 128], BF16,
                                     isOutput=False)
    y = nc.declare_dram_parameter("y", [NQ, DIM], F32, isOutput=True)

    qt_s = nc.dram_tensor("qt_scratch", [DIM, NQ], BF16)
    kt_s = nc.dram_tensor("kt_scratch", [DIM, N], BF16)
    v_s = nc.dram_tensor("v_scratch", [N, DIM], BF16)

    with nc.allow_low_precision(reason="bf16 matmul operands; bf16 stores"), \
         TileContext(nc) as tc:
        with (
            tc.tile_pool(name="const", bufs=1) as constp,
            tc.tile_pool(name="store", bufs=1) as storep,
        ):
            weff_t = constp.tile([128, 1], F32)
            nc.sync.dma_start(out=weff_t, in_=weff[:, :])

            # persistent stores
            argall = storep.tile([H, NQ], F32)
            ustore = storep.tile([128, CIN, NQ], BF16)
            onstore = storep.tile([128, CIN, NQ], BF16)

            # ================= phase A: qkv (bf16) =================
            with (
                tc.tile_pool(name="xbt_p", bufs=2) as xbtp,
                tc.tile_pool(name="wq_p", bufs=1) as wqp,
                tc.tile_pool(name="drain_p", bufs=3) as drp,
                tc.tile_pool(name="psA", bufs=3, space="PSUM") as psA,
                tc.tile_pool(name="psAv", bufs=2, space="PSUM") as psAv,
            ):
                # preload all qkv weights once (per-ci DMAs stall the MMs)
                w_all = wqp.tile([128, CIN, 3 * DIM], BF16)
                nc.sync.dma_start(
                    out=w_all,
                    in_=wqkvt[:, :].rearrange("(c p) n -> p c n", p=128),
                )
                for tq in range(4):  # token quarters of 1024
                    xb = xbtp.tile([128, CIN, 1024], BF16, tag="xb")
                    for h2 in range(2):  # halve the fp32 staging footprint
                        xbf = xbtp.tile([128, CIN, 512], F32, tag="xbf")
                        nc.sync.dma_start(
                            out=xbf,
                            in_=xbt[:, tq * 1024 + h2 * 512:
                                    tq * 1024 + (h2 + 1) * 512]
                            .rearrange("(t p) n -> p t n", p=128),
                        )
                        nc.vector.tensor_copy(
                            xb[:, :, h2 * 512:(h2 + 1) * 512], xbf)
                    for co in range(2 * CIN):  # 0..7 Q, 8..15 K
                        is_q = co < CIN
                        if is_q and tq >= 2:
                            continue
                        ps = psA.tile([128, 1024], F32, tag="ps")
                        for ci in range(CIN):
                            for sb in range(2):
                                nc.tensor.matmul(
                                    ps[:, sb * 512:(sb + 1) * 512],
                                    w_all[:, ci,
                                          co * 128:(co + 1) * 128],
                                    xb[:, ci, sb * 512:(sb + 1) * 512],
                                    start=(ci == 0),
                                    stop=(ci == CIN - 1),
                                )
                        dr = drp.tile([128, 1024], BF16, tag="dr")
                        nc.vector.tensor_copy(dr, ps)
                        dst = qt_s if is_q else kt_s
                        coo = co if is_q else co - CIN
                        nc.sync.dma_start(
                            out=dst[coo * 128:(coo + 1) * 128,
                                    tq * 1024:(tq + 1) * 1024],
                            in_=dr,
                        )
                    for cc in range(DIM // 512):
                        for kt in range(8):
                            psv = psAv.tile([128, 512], F32, tag="psv")
                            for ci in range(CIN):
                                nc.tensor.matmul(
                                    psv,
                                    xb[:, ci, kt * 128:(kt + 1) * 128],
                                    w_all[:, ci, 2 * DIM + cc * 512:
                                          2 * DIM + (cc + 1) * 512],
                                    start=(ci == 0),
                                    stop=(ci == CIN - 1),
                                )
                            drv = drp.tile([128, 512], BF16, tag="drv")
                            if kt % 2 == 0:
                                nc.vector.tensor_copy(drv, psv)
                            else:
                                nc.scalar.activation(
                                    drv, psv,
                                    mybir.ActivationFunctionType.Copy)
                            nc.sync.dma_start(
                                out=v_s[tq * 1024 + kt * 128:
                                        tq * 1024 + (kt + 1) * 128,
                                        cc * 512:(cc + 1) * 512],
                                in_=drv,
                            )

            # ================= phase B: attention =================
            # slots per (head h, qb): slot i = (br=i%2, kt=i//2), strip i%4
            # (K/Q strips duplicated at partitions 64-127 so both slots of a
            #  group hit distinct 32-row PE groups -> fully concurrent S)
            # 2-slot groups x 3 PSUM regions: 3 exp-groups in flight so ACT
            # exps and DVE (Schraudolph) exps genuinely overlap. PE duty is
            # kept high (HAM warm) by folding phase-C work into the B stream.
            GSL = 3  # slots per group (3-slot keeps PE duty high: HAM warm)
            slots_hq = [(i % 2, i // 2, i % 4) for i in range(2 * KT)]
            GPQ = (len(slots_hq) + GSL - 1) // GSL  # 22 (last has 1 slot)
            groups_hq = [slots_hq[GSL * i:GSL * (i + 1)] for i in range(GPQ)]
            # groups whose exp runs on DVE (Schraudolph bf16) instead of ACT
            OFFLOAD_GS = (1, 5, 9, 13, 17)
            A16 = 128.0 / np.log(2.0)
            B16 = 16250.375

            with (
                tc.tile_pool(name="kf_p", bufs=2) as kfp,
                tc.tile_pool(name="qp_p", bufs=2) as qpp,
                tc.tile_pool(name="vh_p", bufs=2) as vhp,
                tc.tile_pool(name="exp_p", bufs=6) as expp,
                tc.tile_pool(name="cp_p", bufs=2) as cpp,
                tc.tile_pool(name="row_p", bufs=2) as rowp,
                tc.tile_pool(name="sd_p", bufs=2) as sdp,
                tc.tile_pool(name="bc_p", bufs=2) as bcp,
                tc.tile_pool(name="psS", bufs=2, space="PSUM") as psS,
                tc.tile_pool(name="psO", bufs=2, space="PSUM") as psO,
            ):
                st = {}
                sched = {}

                def emit_setup_h(h, qb):
                    # K^T/Q^T 64-row head block duplicated to both halves
                    kf = kfp.tile([128, KT, 128], BF16, tag="kf")
                    qp = qpp.tile([128, 512], BF16, tag="qp")
                    for half in range(2):
                        nc.sync.dma_start(
                            out=kf[64 * half:64 * half + 64, :, :],
                            in_=kt_s[h * 64:(h + 1) * 64, :]
                            .rearrange("p (k t) -> p k t", t=128),
                        )
                        nc.sync.dma_start(
                            out=qp[64 * half:64 * half + 64, :],
                            in_=qt_s[h * 64:(h + 1) * 64,
                                     qb * 512:(qb + 1) * 512])
                    vh = vhp.tile([128, KT, 65], BF16, tag="vh")
                    nc.sync.dma_start(
                        out=vh[:, :, 0:VD],
                        in_=v_s[:, h * VD:(h + 1) * VD]
                        .rearrange("(k p) v -> p k v", p=128),
                    )
                    nc.vector.memset(vh[:, :, VD:65], 1.0)
                    st[("kf", h)], st[("qp", h)], st[("vh", h)] = kf, qp, vh

                def emit_S(h, qb, g, gidx):
                    if g == 0:
                        emit_setup_h(h, qb)
                    kf, qp = st[("kf", h)], st[("qp", h)]
                    sps = psS.tile([128, GSL, 512], F32, tag="s")
                    for j, (br, kt, s) in enumerate(groups_hq[g]):
                        nc.tensor.matmul(
                            sps[:, j, :],
                            kf[32 * s:32 * s + 32, kt, :],
                            qp[32 * s:32 * s + 32, :],
                            start=True, stop=True,
                            tile_position=(32 * s, 0),
                        )
                    n = len(groups_hq[g])
                    if g in OFFLOAD_GS:
                        ei = expp.tile([128, GSL, 512], mybir.dt.int16,
                                       tag="e", name="ei")
                        nc.vector.tensor_scalar(
                            out=ei[:, 0:n, :], in0=sps[:, 0:n, :],
                            scalar1=float(A16 * SCALE), scalar2=float(B16),
                            op0=mybir.AluOpType.mult,
                            op1=mybir.AluOpType.add)
                        st[("ex", gidx)] = ei.bitcast(BF16)
                    else:
                        ex = expp.tile([128, GSL, 512], BF16, tag="e",
                                       name="ex")
                        nc.scalar.activation(
                            ex[:, 0:n, :], sps[:, 0:n, :],
                            mybir.ActivationFunctionType.Exp, scale=SCALE,
                        )
                        st[("ex", gidx)] = ex

                def emit_PV(h, qb, g, gidx):
                    vh = st[("vh", h)]
                    ex = st.pop(("ex", gidx))
                    for j, (br, kt, s) in enumerate(groups_hq[g]):
                        if kt == 0:
                            st[("o", br)] = psO.tile(
                                [65, 512], F32, tag="o", name=f"o{br}")
                        nc.tensor.matmul(
                            st[("o", br)],
                            vh[:, kt, 0:65],
                            ex[:, j, :],
                            start=(kt == 0),
                            stop=(kt == KT - 1),
                        )
                    if g == GPQ - 1:
                        emit_drain(h, qb, gidx)

                def emit_drain(h, qb, gidx):
                    # full-tile copies free the 2 PV banks ASAP; ACT does
                    # them so the release is deterministic (DVE queue jitter
                    # stalled the next block's PV by ~4us at p99)
                    o1p = st.pop(("o", 0))
                    o2p = st.pop(("o", 1))
                    sl = slice(qb * 512, (qb + 1) * 512)
                    cp_o1 = cpp.tile([65, 512], F32, tag="co1", name="co1")
                    nc.scalar.activation(
                        cp_o1, o1p, mybir.ActivationFunctionType.Copy)
                    cp_o2 = cpp.tile([65, 512], F32, tag="co2", name="co2")
                    nc.scalar.activation(
                        cp_o2, o2p, mybir.ActivationFunctionType.Copy)
                    # z rows to partition 0 (DVE is lane-locked; DMA moves)
                    z1 = rowp.tile([1, 512], F32, tag="z1", name="z1")
                    nc.sync.dma_start(out=z1, in_=cp_o1[VD:65, :])
                    z2 = rowp.tile([1, 512], F32, tag="z2", name="z2")
                    nc.sync.dma_start(out=z2, in_=cp_o2[VD:65, :])

                    def part1(h=h, sl=sl, cp_o1=cp_o1, cp_o2=cp_o2,
                              z1=z1, z2=z2):
                        rzs = rowp.tile([1, 512], F32, tag="rzs", name="rzs")
                        rz2 = rowp.tile([1, 512], F32, tag="rz2", name="rz2")
                        nc.vector.reciprocal_approx_accurate(
                            out=rz2, in_=z2, scratch=rzs)
                        trow = rowp.tile([1, 512], F32, tag="tr", name="tr")
                        nc.vector.scalar_tensor_tensor(
                            out=trow, in0=z1, scalar=float(lam), in1=rz2,
                            op0=mybir.AluOpType.mult,
                            op1=mybir.AluOpType.mult)
                        tbc = cpp.tile([VD, 512], F32, tag="tbc", name="tbc")
                        nc.gpsimd.partition_broadcast(tbc, trow)
                        st[("t", h, sl.start)] = (tbc, cp_o1, cp_o2, z1)

                    def part2(h=h, qb=qb, sl=sl):
                        tbc, cp_o1, cp_o2, z1 = st.pop(("t", h, sl.start))
                        x1 = cpp.tile([VD, 512], F32, tag="x1", name="x1")
                        nc.vector.tensor_mul(x1, cp_o2[0:VD, :], tbc)
                        u_t = cpp.tile([VD, 512], BF16, tag="ut", name="ut")
                        nc.vector.tensor_sub(u_t, cp_o1[0:VD, :], x1)
                        rsl = slice((h % 2) * VD, (h % 2) * VD + VD)
                        nc.sync.dma_start(
                            out=ustore[rsl, h // 2, sl], in_=u_t)
                        u2 = cpp.tile([VD, 512], F32, tag="u2", name="u2")
                        nc.vector.tensor_mul(u2, u_t, u_t)
                        s2 = cpp.tile([VD, 512], F32, tag="s2", name="s2")
                        nc.gpsimd.partition_all_reduce(
                            s2, u2, channels=VD,
                            reduce_op=bass_isa.ReduceOp.add)
                        st[("u", h, sl.start)] = (s2, z1)

                    def part3(h=h, sl=sl):
                        s2, z1 = st.pop(("u", h, sl.start))
                        ze = rowp.tile([1, 512], F32, tag="ze", name="ze")
                        nc.vector.tensor_scalar_mul(
                            ze, z1, float(EPS ** 0.5))
                        zsq = rowp.tile([1, 512], F32, tag="zq", name="zq")
                        nc.vector.tensor_mul(zsq, ze, ze)
                        arg0 = rowp.tile([1, 512], F32, tag="ar", name="ar")
                        nc.vector.scalar_tensor_tensor(
                            out=arg0, in0=s2[0:1, :],
                            scalar=1.0 / VD, in1=zsq,
                            op0=mybir.AluOpType.mult,
                            op1=mybir.AluOpType.add)
                        nc.sync.dma_start(out=argall[h:h + 1, sl], in_=arg0)

                    sched.setdefault(gidx + 2, []).append(part1)
                    sched.setdefault(gidx + 5, []).append(part2)
                    sched.setdefault(gidx + 8, []).append(part3)

                    if h == H - 1:
                        # whole qb-block drained ~8 groups from now: fold its
                        # phase-C (norm-scale) into the next block's stream
                        def rows_fn(qb=qb, sl=sl):
                            sd = sdp.tile([H, 512], F32, tag="sd", name="sd")
                            nc.scalar.activation(
                                sd, argall[:, sl],
                                mybir.ActivationFunctionType.Sqrt)
                            scr = sdp.tile([H, 512], F32, tag="sc",
                                           name="sc")
                            rrb = sdp.tile([H, 512], F32, tag="rr",
                                           name="rr")
                            nc.vector.reciprocal_approx_accurate(
                                out=rrb, in_=sd, scratch=scr)
                            st[("rrb", qb)] = rrb

                        sched.setdefault(gidx + 10, []).append(rows_fn)
                        for k in range(H):
                            def c_fn(k=k, qb=qb, sl=sl):
                                rrb = st[("rrb", qb)]
                                rowq = rowp.tile([1, 512], F32, tag="rq",
                                                 name="rq")
                                nc.sync.dma_start(
                                    out=rowq, in_=rrb[k:k + 1, :])
                                tbc_rr = bcp.tile([128, 512], F32,
                                                  tag="bc", name="bc")
                                nc.gpsimd.partition_broadcast(tbc_rr, rowq)
                                rsl = slice((k % 2) * VD, (k % 2) * VD + VD)
                                onf = bcp.tile([128, 512], F32, tag="on",
                                               name="on")
                                nc.vector.tensor_mul(
                                    onf[rsl, :], ustore[rsl, k // 2, sl],
                                    tbc_rr[rsl, :])
                                nc.vector.tensor_scalar_mul(
                                    onstore[rsl, k // 2, sl], onf[rsl, :],
                                    weff_t[rsl, :])
                                if k == H - 1:
                                    st.pop(("rrb", qb))

                            sched.setdefault(gidx + 12 + 2 * k,
                                             []).append(c_fn)

                items = [(h, qb, g)
                         for qb in range(4)
                         for h in range(H)
                         for g in range(GPQ)]
                LOOK = 2
                for j in range(LOOK):
                    emit_S(*items[j], j)
                for gidx in range(len(items)):
                    if gidx + LOOK < len(items):
                        emit_S(*items[gidx + LOOK], gidx + LOOK)
                    emit_PV(*items[gidx], gidx)
                    for fn in sched.pop(gidx, []):
                        fn()
                for kk in sorted(sched):
                    for fn in sched[kk]:
                        fn()

            # ============ phase D: proj (bf16, K=128) ============
            # (phase C is folded into the B stream per qb block)
            with (
                tc.tile_pool(name="wp_p", bufs=1) as wpp,
                tc.tile_pool(name="yd_p", bufs=3) as ydp,
                tc.tile_pool(name="psY", bufs=2, space="PSUM") as psY,
            ):
                wpb = wpp.tile([128, CIN, DIM], BF16)
                nc.sync.dma_start(
                    out=wpb,
                    in_=wpbt[:, :].rearrange("v (c n) -> v c n", c=CIN))
                bb = wpp.tile([128, DIM], F32)
                nc.sync.dma_start(out=bb, in_=biasbc[:, :])
                for qt in range(NQ // 128):
                    yps = psY.tile([128, 1024], F32, tag="y")
                    for sb in range(2):
                        for ci in range(CIN):
                            nc.tensor.matmul(
                                yps[:, sb * 512:(sb + 1) * 512],
                                onstore[:, ci, qt * 128:(qt + 1) * 128],
                                wpb[:, ci, sb * 512:(sb + 1) * 512],
                                start=(ci == 0),
                                stop=(ci == CIN - 1),
                            )
                    yd = ydp.tile([128, 1024], F32, tag="yd")
                    nc.vector.tensor_add(yd, yps, bb)
                    nc.sync.dma_start(
                        out=y[qt * 128:(qt + 1) * 128, :], in_=yd)
    nc.finalize()
    return nc


def _make_inputs(x, w_qkv, w_proj, b_proj, sub_norm_w):
    wqkvt = np.ascontiguousarray(
        np.asarray(w_qkv, np.float32).T).astype(ml_dtypes.bfloat16)
    wprojt = np.ascontiguousarray(np.asarray(w_proj, np.float32).T)  # [c, out]
    # proj weights: partition (h%2)*64+vd, col (h//2)*DIM+out
    wpbt = np.ascontiguousarray(
        wprojt.reshape(CIN, 2, VD, DIM).transpose(1, 2, 0, 3)
        .reshape(128, CIN * DIM)).astype(ml_dtypes.bfloat16)
    biasbc = np.ascontiguousarray(
        np.tile(np.asarray(b_proj, np.float32).reshape(1, DIM), (128, 1)))
    # selfat[:, h, :]: [128,128]; row h one-hot -> cols (h%2)*64..+64
    selp = np.zeros((128, H, 128), np.float32)
    for h in range(H):
        po = (h % 2) * VD
        selp[h, h, po:po + VD] = 1.0
    selp = np.ascontiguousarray(
        selp.reshape(128, H * 128)).astype(ml_dtypes.bfloat16)
    weff = np.tile(
        (np.asarray(sub_norm_w, np.float32)
         * (1.0 - LAMBDA_INIT)).reshape(VD, 1), (2, 1))
    weff = np.ascontiguousarray(weff)
    return wqkvt, wpbt, biasbc, weff, selp


def _in_maps(inputs):
    x = np.asarray(inputs["x"], np.float32)
    wqkvt, wpbt, biasbc, weff, selp = _make_inputs(
        x, inputs["w_qkv"], inputs["w_proj"], inputs["b_proj"],
        inputs["sub_norm_w"])
    in_maps = []
    for c in range(NCORES):
        b, half = c // 2, c % 2
        xt = np.asarray(x[b].T)  # [DIM, N]
        if half == 1:  # query rows first
            xt = np.concatenate([xt[:, NQ:], xt[:, :NQ]], axis=1)
        in_maps.append({
            "xbt": np.ascontiguousarray(xt),
            "wqkvt": wqkvt,
            "wpbt": wpbt,
            "biasbc": biasbc,
            "weff": weff,
            "selp": selp,
        })
    return in_maps


def kernel(x, w_qkv, w_proj, b_proj, lambda_q1, lambda_k1, lambda_q2,
           lambda_k2, sub_norm_w):
    lam = float(
        np.exp(np.sum(np.float64(lambda_q1) * np.float64(lambda_k1)))
        - np.exp(np.sum(np.float64(lambda_q2) * np.float64(lambda_k2)))
        + LAMBDA_INIT
    )

    key = round(lam, 12)
    if key not in _CACHE:
        _CACHE[key] = build_nc(lam)
    nc = _CACHE[key]

    in_maps = _in_maps(dict(
        x=x, w_qkv=w_qkv, w_proj=w_proj, b_proj=b_proj, sub_norm_w=sub_norm_w))
    res = run_bass_kernel_spmd(nc, in_maps, list(range(NCORES)))
    out = np.empty((B, N, DIM), np.float32)
    for c in range(NCORES):
        b, half = c // 2, c % 2
        out[b, half * NQ:(half + 1) * NQ, :] = res.results[c]["y"]
    return out
